# revision 6
# baseline (speedup 1.0000x reference)
import sys

sys.path.insert(0, "/opt/trn_rl_repo")

import os
import numpy as np
import ml_dtypes

import concourse.bass as bass
import concourse.mybir as mybir
import concourse.tile as tile
from concourse import bacc
from concourse.bass_utils import run_bass_kernel_spmd
from concourse.masks import make_identity

B, S, D, H = 4, 4096, 1024, 64
QW = 512                      # q-chunk width
NQ = 4                        # q-chunk slots per core
POS = [(0, 3, 4, 7), (1, 2, 5, 6)]   # q-chunk positions per core class
T = (8, 16, 24, 32)           # k-tiles (128 wide) per slot in the uniform graph
NMASK = 8                     # last NMASK tiles of each slot get the causal mask

BF = mybir.dt.bfloat16
F32 = mybir.dt.float32

_cache = {}


def _build():
    nc = bacc.Bacc("TRN2", target_bir_lowering=False, debug=False, num_devices=8)

    xt = nc.dram_tensor("xt", [D, S], BF, kind="ExternalInput").ap()
    xtq = nc.dram_tensor("xtq", [D, NQ * QW], BF, kind="ExternalInput").ap()
    wqk = nc.dram_tensor("wqk", [D, 128], BF, kind="ExternalInput").ap()
    wkv = nc.dram_tensor("wkv", [D, 128], BF, kind="ExternalInput").ap()
    tcol = nc.dram_tensor("tcol", [128, NQ * NMASK], F32, kind="ExternalInput").ap()
    iq = nc.dram_tensor("iq", [128, QW], F32, kind="ExternalInput").ap()
    o = nc.dram_tensor("o", [NQ, H + 1, QW], F32, kind="ExternalOutput").ap()

    ND = D // 128   # 8 d-tiles
    NSC_Q = (NQ * QW) // 512   # 4 moving chunks in q-pass
    NSC_K = S // 512           # 8 moving chunks in kv-pass
    NKT = S // 128             # 32 k tiles

    with tile.TileContext(nc) as tc:
        with (
            tc.tile_pool(name="persist", bufs=1) as pp,
            tc.tile_pool(name="xin", bufs=1) as xp,
            tc.tile_pool(name="estage", bufs=3) as ep,
            tc.tile_pool(name="mstage", bufs=2) as mp,
            tc.tile_pool(name="vstage", bufs=2) as vsp,
            tc.tile_pool(name="ostage", bufs=2) as osp,
            tc.tile_pool(name="zpsum", bufs=2, space="PSUM") as zp,
            tc.tile_pool(name="opsum", bufs=1, space="PSUM") as op_,
            tc.tile_pool(name="projpsum", bufs=2, space="PSUM") as prp,
            tc.tile_pool(name="vtpsum", bufs=1, space="PSUM") as vtp,
        ):
            # ---- persistent tiles ----
            wqk_sb = [pp.tile([128, 128], BF, name=f"wqk{d}", tag=f"wqk{d}") for d in range(ND)]
            wkv_sb = [pp.tile([128, 128], BF, name=f"wkv{d}", tag=f"wkv{d}") for d in range(ND)]
            tcol_sb = pp.tile([128, NQ * NMASK], F32, tag="tcol")
            iq_sb = pp.tile([128, QW], F32, tag="iq")
            ident = pp.tile([64, 64], BF, tag="ident")
            qT = pp.tile([64, NQ * QW], BF, tag="qT")
            kT = pp.tile([64, S], BF, tag="kT")
            vws = pp.tile([128, NKT * (H + 1)], BF, tag="vws")
            xtq_sb = [xp.tile([128, NQ * QW], BF, name=f"xtq{d}", tag=f"xtq{d}") for d in range(ND)]
            xt_sb = [xp.tile([128, S], BF, name=f"xt{d}", tag=f"xt{d}") for d in range(ND)]

            # ---- input DMAs: small/critical first, x in 1024-col slabs ----
            for d in range(ND):
                nc.sync.dma_start(wqk_sb[d][:], wqk[d * 128:(d + 1) * 128, :])
                nc.sync.dma_start(wkv_sb[d][:], wkv[d * 128:(d + 1) * 128, :])
            nc.sync.dma_start(tcol_sb[:], tcol[:])
            nc.sync.dma_start(iq_sb[:], iq[:])
            for c in range(2):
                for d in range(ND):
                    nc.sync.dma_start(
                        xtq_sb[d][:, c * 1024:(c + 1) * 1024],
                        xtq[d * 128:(d + 1) * 128, c * 1024:(c + 1) * 1024])
                for d in range(ND):
                    nc.sync.dma_start(
                        xt_sb[d][:, c * 1024:(c + 1) * 1024],
                        xt[d * 128:(d + 1) * 128, c * 1024:(c + 1) * 1024])
            for c in range(2, 4):
                for d in range(ND):
                    nc.sync.dma_start(
                        xt_sb[d][:, c * 1024:(c + 1) * 1024],
                        xt[d * 128:(d + 1) * 128, c * 1024:(c + 1) * 1024])

            make_identity(nc, ident[:])
            nc.gpsimd.memset(vws[:], 1.0)
            # warm the ACT exp table before the attention phase needs it
            warm = ep.tile([128, 1], BF, tag="warm")
            nc.scalar.activation(warm[:], iq_sb[:, 0:1],
                                 mybir.ActivationFunctionType.Exp)
            # ---- q projection pass: qT[64, 2048] ----
            for sc in range(NSC_Q):
                ps = prp.tile([128, 512], F32, tag="proj")
                for d in range(ND):
                    nc.tensor.matmul(ps[:], wqk_sb[d][:],
                                     xtq_sb[d][:, sc * 512:(sc + 1) * 512],
                                     start=(d == 0), stop=(d == ND - 1))
                nc.vector.tensor_copy(qT[:, sc * 512:(sc + 1) * 512], ps[0:64, :])

            def kv_chunk(sc):
                ps = prp.tile([128, 512], F32, tag="proj", name=f"kvps{sc}")
                for d in range(ND):
                    nc.tensor.matmul(ps[:], wkv_sb[d][:],
                                     xt_sb[d][:, sc * 512:(sc + 1) * 512],
                                     start=(d == 0), stop=(d == ND - 1))
                nc.vector.tensor_copy(kT[:, sc * 512:(sc + 1) * 512], ps[0:64, :])
                vstage = vsp.tile([64, 512], BF, tag="vstage", name=f"vst{sc}")
                nc.vector.tensor_copy(vstage[:], ps[64:128, :])
                for t in range(4):
                    kt_idx = 4 * sc + t
                    vt_ps = vtp.tile([128, 64], BF, tag="vt", name=f"vtps{sc}_{t}")
                    nc.tensor.transpose(vt_ps[:], vstage[:, t * 128:(t + 1) * 128],
                                        ident[:])
                    nc.vector.tensor_copy(
                        vws[:, kt_idx * (H + 1):kt_idx * (H + 1) + H], vt_ps[:])

            def attn_slot(s_):
                ts_ = T[s_]
                ops = op_.tile([H + 1, 512], F32, tag="oacc", name=f"oacc{s_}")
                for jj in range(ts_ // 2):
                    z = zp.tile([128, 1024], F32, tag="z", name=f"z{s_}_{jj}")
                    e = ep.tile([128, 1024], BF, tag="e", name=f"e{s_}_{jj}")
                    for h2 in range(2):
                        j = 2 * jj + h2
                        nc.tensor.matmul(z[:, h2 * 512:(h2 + 1) * 512],
                                         kT[:, j * 128:(j + 1) * 128],
                                         qT[:, s_ * 512:(s_ + 1) * 512],
                                         start=True, stop=True)
                    nc.scalar.activation(e[:], z[:],
                                         mybir.ActivationFunctionType.Exp,
                                         scale=0.125)
                    for h2 in range(2):
                        j = 2 * jj + h2
                        esl = e[:, h2 * 512:(h2 + 1) * 512]
                        if j >= ts_ - NMASK:
                            m = NMASK * s_ + (j - (ts_ - NMASK))
                            msk = mp.tile([128, 512], BF, tag="msk",
                                          name=f"msk{s_}_{j}")
                            nc.vector.tensor_scalar(msk[:], iq_sb[:],
                                                    tcol_sb[:, m:m + 1], None,
                                                    mybir.AluOpType.is_ge)
                            nc.vector.tensor_tensor(esl, esl, msk[:],
                                                    mybir.AluOpType.mult)
                        nc.tensor.matmul(ops[:],
                                         vws[:, j * (H + 1):(j + 1) * (H + 1)],
                                         esl, start=(j == 0),
                                         stop=(j == ts_ - 1))
                osb = osp.tile([H + 1, 512], F32, tag="osb", name=f"osb{s_}")
                nc.vector.tensor_copy(osb[:], ops[:])
                nc.sync.dma_start(o[s_], osb[:])

            # interleave kv chunks with attention slots: slot s needs k-tiles
            # [0, T[s]) = kv chunks [0, T[s]/4)
            kv_chunk(0)
            kv_chunk(1)
            attn_slot(0)
            kv_chunk(2)
            kv_chunk(3)
            attn_slot(1)
            kv_chunk(4)
            kv_chunk(5)
            attn_slot(2)
            kv_chunk(6)
            kv_chunk(7)
            attn_slot(3)

    nc.compile()
    return nc


def _get_nc():
    if "nc" not in _cache:
        _cache["nc"] = _build()
    return _cache["nc"]


def kernel(x, Wk, Wq, Wv):
    x = np.asarray(x, dtype=np.float32)
    Wk = np.asarray(Wk, dtype=np.float32)
    Wq = np.asarray(Wq, dtype=np.float32)
    Wv = np.asarray(Wv, dtype=np.float32)

    nc = _get_nc()

    wqk_np = np.concatenate([Wq, Wk], axis=1).astype(ml_dtypes.bfloat16)
    wkv_np = np.concatenate([Wk, Wv], axis=1).astype(ml_dtypes.bfloat16)
    iq_np = np.broadcast_to(np.arange(QW, dtype=np.float32), (128, QW)).copy()

    xt_b = [np.ascontiguousarray(x[b].T).astype(ml_dtypes.bfloat16)
            for b in range(B)]

    in_maps = []
    for c in range(8):
        b, cls = c >> 1, c & 1
        pos = POS[cls]
        xtq_np = np.concatenate(
            [xt_b[b][:, p * QW:(p + 1) * QW] for p in pos], axis=1)
        tcol_np = np.zeros((128, NQ * NMASK), np.float32)
        krange = np.arange(128, dtype=np.float32)
        for s_ in range(NQ):
            for jj in range(NMASK):
                j = T[s_] - NMASK + jj
                tcol_np[:, NMASK * s_ + jj] = krange + 128.0 * j - 512.0 * pos[s_]
        in_maps.append({
            "xt": xt_b[b],
            "xtq": np.ascontiguousarray(xtq_np),
            "wqk": wqk_np,
            "wkv": wkv_np,
            "tcol": tcol_np,
            "iq": iq_np,
        })

    trace = bool(int(os.environ.get("KERNEL_TRACE", "0")))
    res = run_bass_kernel_spmd(nc, in_maps, core_ids=list(range(8)), trace=trace)
    _cache["last_result"] = res

    out = np.zeros((B, S, H), np.float32)
    for c in range(8):
        b, cls = c >> 1, c & 1
        oc = res.results[c]["o"]          # [NQ, 65, 512]
        for s_, p in enumerate(POS[cls]):
            num = oc[s_, 0:H, :]          # [64, 512]
            den = oc[s_, H, :]            # [512]
            out[b, p * QW:(p + 1) * QW, :] = (num / den[None, :]).T
    return out
